# revision 2
# baseline (speedup 1.0000x reference)
"""Multi-head self-attention Bass/Tile kernel for Trainium2, SPMD over 8 cores.

Problem: B=2, T=4096, D=768, H=12, HD=64 dense MHSA (full TxT scores,
key-padding mask, softmax, out-proj with bias).

Sharding: core c handles batch b=c//4 and query slice q0=(c%4)*1024 for all
12 heads.  No collectives: each core computes a disjoint [768, 1024] slice
of the (transposed) output; the host gathers.

Key optimization vs the naive version: the key-padding mask knocks out ~half
of the 4096 keys (mask==True -> -inf score -> zero attention weight).  The
host compacts the keys (gathers unmasked columns of x^T, zero-padding to a
multiple of 128) so K/V projection, scores, exp and AV all run on ~2176
instead of 4096 keys.  Padding keys get a -1e9 softmax bias so they
contribute exactly zero to both numerator and denominator.

All matmuls contract over the partition dim ("transposed" dataflow:
features on partitions, tokens free):
  phase A: QKV projection, all SBUF-resident (no DRAM staging).
           Q^T packed two heads per [128, 1024] tile; K^T packed the same
           way as [128, n_pad] tiles; V' natural layout [n_pad, 12*(64+1)]
           with a ones column per head (softmax denominator falls out of
           the AV matmul).
  phase B: per head-pair (2j, 2j+1), per key-tile kt:
           S[128k, 2*1024q] = K^T.T @ Q^T (4 matmuls into one PSUM window),
           P = exp(S/8 + maskbias_k) in ONE wide ACT instruction (2048 free),
           O'[65, 512] += V'_kt.T @ P (PSUM accumulation over kt).
           Normalize O = O'[0:64] * bcast(1/O'[64]) (gpsimd partition
           broadcast + DVE multiply; no PE involvement).
  phase C: out^T[768, 1024] = Wp^T.T @ O^T + b, DMA out.
"""

import functools
import numpy as np

import concourse.bass as bass
import concourse.mybir as mybir
import concourse.tile as tile
from concourse import bacc
from concourse.bass2jax import (
    _bass_exec_p,
    install_neuronx_cc_hook,
    partition_id_tensor,
)

F32R = mybir.dt.float32r
F32 = mybir.dt.float32
BF16 = mybir.dt.bfloat16
USE_BF16 = True
MMDT = BF16 if USE_BF16 else F32R
AF = mybir.ActivationFunctionType

B, T, D = 2, 4096, 768
H, HD = 12, 64
N_CORES = 8
CORES_PER_B = 4
QS = T // CORES_PER_B          # 1024 query tokens per core
NB = 1e9                        # mask bias magnitude
DT = D // 128                   # 6 d-tiles
QC = QS // 512                  # 2 query chunks of 512
N_PAD_DEFAULT = 2176


def _n_pad_from_mask(mask) -> int:
    n = int(np.max(np.sum(~np.asarray(mask, bool), axis=1)))
    n = max(n, 128)
    return (n + 127) // 128 * 128


@functools.lru_cache(maxsize=None)
def build_program(reps: int = 1, n_pad: int = N_PAD_DEFAULT):
    nc = bacc.Bacc("TRN2", target_bir_lowering=False, debug=False,
                   num_devices=N_CORES)

    xTq = nc.dram_tensor("xTq", [D, QS], MMDT, kind="ExternalInput").ap()
    xTc = nc.dram_tensor("xTc", [D, n_pad], MMDT, kind="ExternalInput").ap()
    wqT = nc.dram_tensor("wqT", [D, D], MMDT, kind="ExternalInput").ap()
    wkT = nc.dram_tensor("wkT", [D, D], MMDT, kind="ExternalInput").ap()
    wvT = nc.dram_tensor("wvT", [D, D], MMDT, kind="ExternalInput").ap()
    wpT = nc.dram_tensor("wpT", [D, D], MMDT, kind="ExternalInput").ap()
    bp = nc.dram_tensor("bp", [128, DT], F32, kind="ExternalInput").ap()
    mbc = nc.dram_tensor("mbc", [128, n_pad // 128], F32,
                         kind="ExternalInput").ap()
    onesc = nc.dram_tensor("onesc", [128, H], MMDT, kind="ExternalInput").ap()
    outT = nc.dram_tensor("outT", [D, QS], F32, kind="ExternalOutput").ap()

    with tile.TileContext(nc) as tc, nc.allow_low_precision(
            reason="bf16 matmul pipeline"):
        _body(nc, tc, reps, n_pad, xTq, xTc, wqT, wkT, wvT, wpT, bp, mbc,
              onesc, outT)
    nc.compile()
    return nc


def _body(nc, tc, reps, n_pad, xTq, xTc, wqT, wkT, wvT, wpT, bp, mbc,
          onesc, outT):
    from contextlib import ExitStack

    KTc = n_pad // 128

    with ExitStack() as root:
        const = root.enter_context(tc.tile_pool(name="const", bufs=1))
        mb_sb = const.tile([128, KTc], F32, tag="mb")
        nc.sync.dma_start(mb_sb[:], mbc[:])
        bp_sb = const.tile([128, DT], F32, tag="bp")
        nc.sync.dma_start(bp_sb[:], bp[:])
        onesr = const.tile([128, H], MMDT, tag="onesr")
        nc.sync.dma_start(onesr[:], onesc[:])

        # long-lived packed tiles
        qt_pool = root.enter_context(tc.tile_pool(name="qt", bufs=1))
        ksb_pool = root.enter_context(tc.tile_pool(name="ksb", bufs=1))
        vsb_pool = root.enter_context(tc.tile_pool(name="vsb", bufs=1))
        ot_pool = root.enter_context(tc.tile_pool(name="ot", bufs=1))
        wp_pool = root.enter_context(tc.tile_pool(name="wp", bufs=1))

        wp_sb = []
        for j in range(DT):
            wpt = wp_pool.tile([128, D], MMDT, tag=f"wp{j}")
            nc.sync.dma_start(wpt[:], wpT[j * 128:(j + 1) * 128, :])
            wp_sb.append(wpt)

        def emit_once():
            qts, ksb, vsb = _phase_a(nc, tc, qt_pool, ksb_pool, vsb_pool,
                                     n_pad, xTq, xTc, wqT, wkT, wvT, onesr)
            ots = _phase_b(nc, tc, ot_pool, n_pad, qts, ksb, vsb, mb_sb)
            _phase_c(nc, tc, ots, wp_sb, bp_sb, outT)

        if reps == 1:
            emit_once()
        elif reps < 0:
            for _ in range(-reps):
                emit_once()
        else:
            with tc.For_i(0, reps, 1):
                emit_once()


def _phase_a(nc, tc, qt_pool, ksb_pool, vsb_pool, n_pad, xTq, xTc,
             wqT, wkT, wvT, onesr):
    from contextlib import ExitStack

    KTc = n_pad // 128
    qts, ksb, vsb = [], [], []
    with ExitStack() as s:
        w_pool = s.enter_context(tc.tile_pool(name="wqkv", bufs=1))
        xq_pool = s.enter_context(tc.tile_pool(name="xq", bufs=1))
        xc_pool = s.enter_context(tc.tile_pool(name="xc", bufs=1))
        qps_pool = s.enter_context(
            tc.tile_pool(name="qps", bufs=2, space="PSUM"))
        kvps_pool = s.enter_context(
            tc.tile_pool(name="kvps", bufs=2, space="PSUM"))

        wq_sb, wk_sb, wv_sb, xq_sb, xc_sb = [], [], [], [], []
        for d in range(DT):
            wq = w_pool.tile([128, D], MMDT, tag=f"wq{d}")
            nc.sync.dma_start(wq[:], wqT[d * 128:(d + 1) * 128, :])
            wq_sb.append(wq)
            wk = w_pool.tile([128, D], MMDT, tag=f"wk{d}")
            nc.sync.dma_start(wk[:], wkT[d * 128:(d + 1) * 128, :])
            wk_sb.append(wk)
            wv = w_pool.tile([128, D], MMDT, tag=f"wv{d}")
            nc.sync.dma_start(wv[:], wvT[d * 128:(d + 1) * 128, :])
            wv_sb.append(wv)
            xq = xq_pool.tile([128, QS], MMDT, tag=f"xq{d}")
            nc.sync.dma_start(xq[:], xTq[d * 128:(d + 1) * 128, :])
            xq_sb.append(xq)
            xc = xc_pool.tile([128, n_pad], MMDT, tag=f"xc{d}")
            nc.sync.dma_start(xc[:], xTc[d * 128:(d + 1) * 128, :])
            xc_sb.append(xc)

        # --- Q^T: 6 tiles [128, QS], two heads per tile ---
        for j in range(DT):
            qt = qt_pool.tile([128, QS], MMDT, tag=f"qt{j}")
            for c in range(QC):
                ps = qps_pool.tile([128, 512], F32, tag="qps")
                for d in range(DT):
                    nc.tensor.matmul(
                        ps[:], wq_sb[d][:, j * 128:(j + 1) * 128],
                        xq_sb[d][:, c * 512:(c + 1) * 512],
                        start=(d == 0), stop=(d == DT - 1))
                nc.vector.tensor_copy(qt[:, c * 512:(c + 1) * 512], ps[:])
            qts.append(qt)

        # --- K^T: 6 tiles [128, n_pad], two heads per tile ---
        chunks = [(o, min(512, n_pad - o)) for o in range(0, n_pad, 512)]
        for j in range(DT):
            kt_t = ksb_pool.tile([128, n_pad], MMDT, tag=f"ksb{j}")
            for (o, w) in chunks:
                ps = kvps_pool.tile([128, 512], F32, tag="kps")
                for d in range(DT):
                    nc.tensor.matmul(
                        ps[:, 0:w], wk_sb[d][:, j * 128:(j + 1) * 128],
                        xc_sb[d][:, o:o + w],
                        start=(d == 0), stop=(d == DT - 1))
                nc.vector.tensor_copy(kt_t[:, o:o + w], ps[:, 0:w])
            ksb.append(kt_t)

        # --- V' natural layout [128 keys, 12*(64+1)] per key-tile ---
        for tt in range(KTc):
            ps = kvps_pool.tile([128, D], F32, tag="vps")
            for d in range(DT):
                lhs = xc_sb[d][:, tt * 128:(tt + 1) * 128]
                nc.tensor.matmul(ps[:, 0:512], lhs, wv_sb[d][:, 0:512],
                                 start=(d == 0), stop=(d == DT - 1),
                                 skip_group_check=True)
                nc.tensor.matmul(ps[:, 512:768], lhs, wv_sb[d][:, 512:768],
                                 start=(d == 0), stop=(d == DT - 1),
                                 skip_group_check=True)
            vt = vsb_pool.tile([128, H * (HD + 1)], MMDT, tag=f"vsb{tt}")
            vt3 = vt[:].rearrange("p (h s) -> p h s", s=HD + 1)
            nc.vector.tensor_copy(
                vt3[:, :, 0:HD],
                ps[:].rearrange("p (h s) -> p h s", s=HD))
            nc.vector.tensor_copy(
                vt3[:, :, HD:HD + 1],
                onesr[:].rearrange("p (h o) -> p h o", o=1))
            vsb.append(vt)
    return qts, ksb, vsb


def _phase_b(nc, tc, ot_pool, n_pad, qts, ksb, vsb, mb_sb):
    from contextlib import ExitStack

    KTc = n_pad // 128
    ots = []
    with ExitStack() as s:
        p_pool = s.enter_context(tc.tile_pool(name="p", bufs=3))
        nrm_pool = s.enter_context(tc.tile_pool(name="nrm", bufs=2))
        sp_pool = s.enter_context(
            tc.tile_pool(name="sp", bufs=1, space="PSUM"))
        op_pool = s.enter_context(
            tc.tile_pool(name="op", bufs=1, space="PSUM"))

        for j in range(DT):        # head pair (2j, 2j+1)
            ops = [[op_pool.tile([65, 512], F32, tag=f"op{hi}{c}",
                                 name=f"op{hi}{c}")
                    for c in range(QC)] for hi in range(2)]
            for kt in range(KTc):
                sp = sp_pool.tile([128, 2 * QS], F32, tag="sp")
                for hi in range(2):
                    poff = hi * 64
                    for c in range(QC):
                        nc.tensor.matmul(
                            sp[:, hi * QS + c * 512: hi * QS + (c + 1) * 512],
                            ksb[j][poff:poff + 64, kt * 128:(kt + 1) * 128],
                            qts[j][poff:poff + 64, c * 512:(c + 1) * 512],
                            start=True, stop=True, skip_group_check=True)
                p = p_pool.tile([128, 2 * QS], MMDT, tag="p")
                nc.scalar.activation(p[:], sp[:], AF.Exp,
                                     bias=mb_sb[:, kt:kt + 1], scale=0.125)
                for hi in range(2):
                    h = 2 * j + hi
                    for c in range(QC):
                        nc.tensor.matmul(
                            ops[hi][c][:],
                            vsb[kt][:, h * (HD + 1):(h + 1) * (HD + 1)],
                            p[:, hi * QS + c * 512: hi * QS + (c + 1) * 512],
                            start=(kt == 0), stop=(kt == KTc - 1))

            ot = ot_pool.tile([128, QS], MMDT, tag=f"ot{j}")
            for hi in range(2):
                for c in range(QC):
                    recip = nrm_pool.tile([1, 512], F32, tag="recip")
                    nc.vector.reciprocal(recip[:], ops[hi][c][64:65, :])
                    bc_sb = nrm_pool.tile([64, 512], F32, tag="bc_sb")
                    nc.gpsimd.partition_broadcast(bc_sb[:], recip[:])
                    nc.vector.tensor_mul(
                        ot[hi * 64:(hi + 1) * 64, c * 512:(c + 1) * 512],
                        ops[hi][c][0:64, :], bc_sb[:])
            ots.append(ot)
    return ots


def _phase_c(nc, tc, ots, wp_sb, bp_sb, outT):
    from contextlib import ExitStack

    with ExitStack() as s:
        ost_pool = s.enter_context(tc.tile_pool(name="ost", bufs=3))
        pps_pool = s.enter_context(
            tc.tile_pool(name="pps", bufs=2, space="PSUM"))

        for m in range(DT):
            for c in range(QC):
                ps = pps_pool.tile([128, 512], F32, tag="pps")
                for j in range(DT):
                    nc.tensor.matmul(
                        ps[:], wp_sb[j][:, m * 128:(m + 1) * 128],
                        ots[j][:, c * 512:(c + 1) * 512],
                        start=(j == 0), stop=(j == DT - 1))
                ost = ost_pool.tile([128, 512], F32, tag="ost")
                nc.vector.tensor_scalar_add(ost[:], ps[:], bp_sb[:, m:m + 1])
                nc.sync.dma_start(
                    outT[m * 128:(m + 1) * 128, c * 512:(c + 1) * 512],
                    ost[:])


# ---------------------------------------------------------------- host side

@functools.lru_cache(maxsize=None)
def _get_runner(reps: int = 1, n_pad: int = N_PAD_DEFAULT):
    import jax
    from jax.sharding import Mesh, PartitionSpec
    from jax.experimental.shard_map import shard_map

    nc = build_program(reps, n_pad)
    install_neuronx_cc_hook()
    partition_name = (nc.partition_id_tensor.name
                      if nc.partition_id_tensor else None)
    in_names, out_names, out_avals, out_shapes = [], [], [], []
    for alloc in nc.m.functions[0].allocations:
        if not isinstance(alloc, mybir.MemoryLocationSet):
            continue
        name = alloc.memorylocations[0].name
        if alloc.kind == "ExternalInput":
            if name != partition_name:
                in_names.append(name)
        elif alloc.kind == "ExternalOutput":
            out_names.append(name)
            shape = tuple(alloc.tensor_shape)
            dtype = mybir.dt.np(alloc.dtype)
            out_avals.append(jax.core.ShapedArray(shape, dtype))
            out_shapes.append((shape, dtype))
    n_params = len(in_names)
    n_outs = len(out_avals)
    all_in_names = list(in_names) + list(out_names)
    if partition_name is not None:
        all_in_names.append(partition_name)
    donate = tuple(range(n_params, n_params + n_outs))

    def _bodyf(*args):
        operands = list(args)
        if partition_name is not None:
            operands.append(partition_id_tensor())
        outs = _bass_exec_p.bind(
            *operands,
            out_avals=tuple(out_avals),
            in_names=tuple(all_in_names),
            out_names=tuple(out_names),
            lowering_input_output_aliases=(),
            sim_require_finite=True,
            sim_require_nnan=True,
            nc=nc,
        )
        return tuple(outs)

    devices = jax.devices()[:N_CORES]
    mesh = Mesh(np.asarray(devices), ("core",))
    in_specs = (PartitionSpec("core"),) * (n_params + n_outs)
    out_specs = (PartitionSpec("core"),) * len(out_names)
    sharded = jax.jit(
        shard_map(_bodyf, mesh=mesh, in_specs=in_specs, out_specs=out_specs,
                  check_rep=False),
        donate_argnums=donate, keep_unused=True,
    )

    def run(in_maps):
        import jax as _jax
        per_core = [[np.asarray(m[n]) for n in in_names] for m in in_maps]
        concat_in = [np.concatenate([per_core[c][i] for c in range(N_CORES)],
                                    axis=0) for i in range(n_params)]
        concat_zeros = [np.zeros((N_CORES * s[0], *s[1:]), dt)
                        for (s, dt) in out_shapes]
        out_arrs = sharded(*concat_in, *concat_zeros)
        _jax.block_until_ready(out_arrs)
        return [
            {name: np.asarray(out_arrs[i]).reshape(
                N_CORES, *out_shapes[i][0])[c]
             for i, name in enumerate(out_names)}
            for c in range(N_CORES)
        ]

    return run


def make_in_maps(x, mask, w_qkv, w_proj, b_proj):
    import ml_dtypes
    mm_np = ml_dtypes.bfloat16 if USE_BF16 else np.float32
    x = np.asarray(x, np.float32)
    mask = np.asarray(mask, bool)
    w_qkv = np.asarray(w_qkv, np.float32)
    w_proj = np.asarray(w_proj, np.float32)
    b_proj = np.asarray(b_proj, np.float32)

    n_pad = _n_pad_from_mask(mask)
    KTc = n_pad // 128

    wqT = np.ascontiguousarray(w_qkv[0:D].T).astype(mm_np)
    wkT = np.ascontiguousarray(w_qkv[D:2 * D].T).astype(mm_np)
    wvT = np.ascontiguousarray(w_qkv[2 * D:3 * D].T).astype(mm_np)
    wpT = np.ascontiguousarray(w_proj.T).astype(mm_np)
    bp = np.ascontiguousarray(b_proj.reshape(DT, 128).T)
    onesc = np.ones((128, H), mm_np)

    xTs, xTcs, mbs = [], [], []
    for b in range(B):
        xT = np.ascontiguousarray(x[b].T).astype(mm_np)
        xTs.append(xT)
        idx = np.flatnonzero(~mask[b])
        nb = len(idx)
        xTc = np.zeros((D, n_pad), mm_np)
        xTc[:, :nb] = xT[:, idx]
        xTcs.append(xTc)
        mb = np.full(n_pad, -np.float32(NB), np.float32)
        mb[:nb] = 0.0
        mbs.append(np.ascontiguousarray(mb.reshape(KTc, 128).T))

    in_maps = []
    for c in range(N_CORES):
        b, qi = divmod(c, CORES_PER_B)
        q0 = qi * QS
        in_maps.append({
            "xTq": np.ascontiguousarray(xTs[b][:, q0:q0 + QS]),
            "xTc": xTcs[b],
            "wqT": wqT, "wkT": wkT, "wvT": wvT, "wpT": wpT,
            "bp": bp, "mbc": mbs[b], "onesc": onesc,
        })
    return in_maps


def assemble_output(results):
    out = np.empty((B, T, D), np.float32)
    for c in range(N_CORES):
        b, qi = divmod(c, CORES_PER_B)
        q0 = qi * QS
        out[b, q0:q0 + QS, :] = results[c]["outT"].T
    return out


def kernel(x, mask, w_qkv, w_proj, b_proj):
    n_pad = _n_pad_from_mask(mask)
    run = _get_runner(1, n_pad)
    in_maps = make_in_maps(x, mask, w_qkv, w_proj, b_proj)
    results = run(in_maps)
    return assemble_output(results)


# revision 7
# speedup vs baseline: 1.3767x; 1.3767x over previous
"""Multi-head self-attention Bass/Tile kernel for Trainium2, SPMD over 8 cores.

Problem: B=2, T=4096, D=768, H=12, HD=64 dense MHSA (full TxT scores,
key-padding mask, softmax, out-proj with bias).

Sharding: core c handles batch b=c//4 and query slice q0=(c%4)*1024 for all
12 heads.  No collectives: each core computes a disjoint [768, 1024] slice
of the (transposed) output; the host gathers.

Key optimization vs the naive version: the key-padding mask knocks out ~half
of the 4096 keys (mask==True -> -inf score -> zero attention weight).  The
host compacts the keys (gathers unmasked columns of x^T, zero-padding to a
multiple of 128) so K/V projection, scores, exp and AV all run on ~2176
instead of 4096 keys.  Padding keys get a -1e9 softmax bias so they
contribute exactly zero to both numerator and denominator.

All matmuls contract over the partition dim ("transposed" dataflow:
features on partitions, tokens free):
  phase A: QKV projection, all SBUF-resident (no DRAM staging).
           Q^T packed two heads per [128, 1024] tile; K^T packed the same
           way as [128, n_pad] tiles; V' natural layout [n_pad, 12*(64+1)]
           with a ones column per head (softmax denominator falls out of
           the AV matmul).
  phase B: per head-pair (2j, 2j+1), per key-tile kt:
           S[128k, 2*1024q] = K^T.T @ Q^T (4 matmuls into one PSUM window),
           P = exp(S/8 + maskbias_k) in ONE wide ACT instruction (2048 free),
           O'[65, 512] += V'_kt.T @ P (PSUM accumulation over kt).
           Normalize O = O'[0:64] * bcast(1/O'[64]) (gpsimd partition
           broadcast + DVE multiply; no PE involvement).
  phase C: out^T[768, 1024] = Wp^T.T @ O^T + b, DMA out.
"""

import functools
import numpy as np

import concourse.bass as bass
import concourse.mybir as mybir
import concourse.tile as tile
from concourse import bacc
from concourse.bass2jax import (
    _bass_exec_p,
    install_neuronx_cc_hook,
    partition_id_tensor,
)

F32R = mybir.dt.float32r
F32 = mybir.dt.float32
BF16 = mybir.dt.bfloat16
USE_BF16 = True
MMDT = BF16 if USE_BF16 else F32R
AF = mybir.ActivationFunctionType

B, T, D = 2, 4096, 768
H, HD = 12, 64
N_CORES = 8
CORES_PER_B = 4
QS = T // CORES_PER_B          # 1024 query tokens per core
NB = 1e9                        # mask bias magnitude
DT = D // 128                   # 6 d-tiles
QC = QS // 512                  # 2 query chunks of 512
N_PAD_DEFAULT = 2176


def _n_pad_from_mask(mask) -> int:
    n = int(np.max(np.sum(~np.asarray(mask, bool), axis=1)))
    n = max(n, 128)
    return (n + 127) // 128 * 128


@functools.lru_cache(maxsize=None)
def build_program(reps: int = 1, n_pad: int = N_PAD_DEFAULT):
    nc = bacc.Bacc("TRN2", target_bir_lowering=False, debug=False,
                   num_devices=N_CORES)

    xTq = nc.dram_tensor("xTq", [D, QS], MMDT, kind="ExternalInput").ap()
    xTc = nc.dram_tensor("xTc", [D, n_pad], MMDT, kind="ExternalInput").ap()
    wqT = nc.dram_tensor("wqT", [D, D], MMDT, kind="ExternalInput").ap()
    wkT = nc.dram_tensor("wkT", [D, D], MMDT, kind="ExternalInput").ap()
    wvT = nc.dram_tensor("wvT", [D, D], MMDT, kind="ExternalInput").ap()
    wpT = nc.dram_tensor("wpT", [D, D], MMDT, kind="ExternalInput").ap()
    bp = nc.dram_tensor("bp", [128, DT], F32, kind="ExternalInput").ap()
    mbc = nc.dram_tensor("mbc", [128, n_pad // 128], F32,
                         kind="ExternalInput").ap()
    onesc = nc.dram_tensor("onesc", [128, H], MMDT, kind="ExternalInput").ap()
    outT = nc.dram_tensor("outT", [D, QS], F32, kind="ExternalOutput").ap()

    with tile.TileContext(nc) as tc, nc.allow_low_precision(
            reason="bf16 matmul pipeline"):
        _body(nc, tc, reps, n_pad, xTq, xTc, wqT, wkT, wvT, wpT, bp, mbc,
              onesc, outT)
    nc.compile()
    return nc


def _body(nc, tc, reps, n_pad, xTq, xTc, wqT, wkT, wvT, wpT, bp, mbc,
          onesc, outT):
    from contextlib import ExitStack

    KTc = n_pad // 128

    with ExitStack() as root:
        const = root.enter_context(tc.tile_pool(name="const", bufs=1))
        mb_sb = const.tile([128, KTc], F32, tag="mb")
        nc.sync.dma_start(mb_sb[:], mbc[:])
        bp_sb = const.tile([128, DT], F32, tag="bp")
        nc.sync.dma_start(bp_sb[:], bp[:])
        onesr = const.tile([128, H], MMDT, tag="onesr")
        nc.sync.dma_start(onesr[:], onesc[:])

        # long-lived packed tiles
        qt_pool = root.enter_context(tc.tile_pool(name="qt", bufs=1))
        ksb_pool = root.enter_context(tc.tile_pool(name="ksb", bufs=1))
        vsb_pool = root.enter_context(tc.tile_pool(name="vsb", bufs=1))
        ot_pool = root.enter_context(tc.tile_pool(name="ot", bufs=1))
        wp_pool = root.enter_context(tc.tile_pool(name="wp", bufs=1))

        wp_sb = []
        for j in range(DT):
            wpt = wp_pool.tile([128, D], MMDT, tag=f"wp{j}")
            nc.sync.dma_start(wpt[:], wpT[j * 128:(j + 1) * 128, :])
            wp_sb.append(wpt)

        def emit_once():
            ots = _phase_ab(nc, tc, qt_pool, ksb_pool, vsb_pool, ot_pool,
                            n_pad, xTq, xTc, wqT, wkT, wvT, onesr, mb_sb)
            _phase_c(nc, tc, ots, wp_sb, bp_sb, outT)

        if reps == 1:
            emit_once()
        elif reps < 0:
            for _ in range(-reps):
                emit_once()
        else:
            with tc.For_i(0, reps, 1):
                emit_once()


def _phase_ab(nc, tc, qt_pool, ksb_pool, vsb_pool, ot_pool, n_pad,
              xTq, xTc, wqT, wkT, wvT, onesr, mb_sb):
    """Fused QKV projection + attention with interleaved emission.

    Emission order: Q, K0, then per head-pair j: the two heads' kt loops,
    with V' production interleaved just-in-time into head 0's loop and
    K_{j+1} production spread into pair j's loops.  This gets the ACT
    engine (exp stream, the phase-B bottleneck) started ~25us into the
    program instead of after the whole projection phase.

    PSUM budget: one shared pool for all projection/score outputs
    (tag "sp", [128,1024] f32 = 2 banks x 2 bufs) + AV accumulators
    (2 tags [65,512] x 2 bufs = 4 banks) = 8 banks exactly.
    """
    from contextlib import ExitStack

    KTc = n_pad // 128
    qts, ksb, vsb = [], [None] * DT, [None] * KTc
    ots = []
    with ExitStack() as s:
        w_pool = s.enter_context(tc.tile_pool(name="wqkv", bufs=1))
        xq_pool = s.enter_context(tc.tile_pool(name="xq", bufs=1))
        xc_pool = s.enter_context(tc.tile_pool(name="xc", bufs=1))

        wq_sb, wk_sb, wv_sb, xq_sb, xc_sb = [], [], [], [], []
        for d in range(DT):
            wq = w_pool.tile([128, D], MMDT, tag=f"wq{d}")
            nc.sync.dma_start(wq[:], wqT[d * 128:(d + 1) * 128, :])
            wq_sb.append(wq)
            wk = w_pool.tile([128, D], MMDT, tag=f"wk{d}")
            nc.sync.dma_start(wk[:], wkT[d * 128:(d + 1) * 128, :])
            wk_sb.append(wk)
            wv = w_pool.tile([128, D], MMDT, tag=f"wv{d}")
            nc.sync.dma_start(wv[:], wvT[d * 128:(d + 1) * 128, :])
            wv_sb.append(wv)
            xq = xq_pool.tile([128, QS], MMDT, tag=f"xq{d}")
            nc.sync.dma_start(xq[:], xTq[d * 128:(d + 1) * 128, :])
            xq_sb.append(xq)
            xc = xc_pool.tile([128, n_pad], MMDT, tag=f"xc{d}")
            nc.sync.dma_start(xc[:], xTc[d * 128:(d + 1) * 128, :])
            xc_sb.append(xc)

        # --- Q^T: 6 tiles [128, QS], two heads per tile ---
        with tc.tile_pool(name="qps", bufs=2, space="PSUM") as qps_pool:
            for j in range(DT):
                qt = qt_pool.tile([128, QS], MMDT, tag=f"qt{j}")
                for c in range(QC):
                    ps = qps_pool.tile([128, 512], F32, tag="qps")
                    for d in range(DT):
                        nc.tensor.matmul(
                            ps[:], wq_sb[d][:, j * 128:(j + 1) * 128],
                            xq_sb[d][:, c * 512:(c + 1) * 512],
                            start=(d == 0), stop=(d == DT - 1))
                    nc.vector.tensor_copy(qt[:, c * 512:(c + 1) * 512], ps[:])
                qts.append(qt)

        p_pool = s.enter_context(tc.tile_pool(name="p", bufs=3))
        nrm_pool = s.enter_context(tc.tile_pool(name="nrm", bufs=2))
        sp_pool = s.enter_context(
            tc.tile_pool(name="sp", bufs=2, space="PSUM"))
        op_pool = s.enter_context(
            tc.tile_pool(name="op", bufs=2, space="PSUM"))

        # K_j production in chunks through the shared sp psum pool.
        def k_chunks(j):
            kt_t = ksb_pool.tile([128, n_pad], MMDT, tag=f"ksb{j}")
            ksb[j] = kt_t
            for o in range(0, n_pad, QS):
                w = min(QS, n_pad - o)

                def emit(o=o, w=w, j=j, kt_t=kt_t):
                    ps = sp_pool.tile([128, QS], F32, tag="sp")
                    for half in range(0, w, 512):
                        hw_ = min(512, w - half)
                        for d in range(DT):
                            nc.tensor.matmul(
                                ps[:, half:half + hw_],
                                wk_sb[d][:, j * 128:(j + 1) * 128],
                                xc_sb[d][:, o + half:o + half + hw_],
                                start=(d == 0), stop=(d == DT - 1),
                                skip_group_check=True)
                    nc.vector.tensor_copy(kt_t[:, o:o + w], ps[:, 0:w])
                yield emit

        def emit_v(tt):
            ps = sp_pool.tile([128, QS], F32, tag="sp")
            for d in range(DT):
                lhs = xc_sb[d][:, tt * 128:(tt + 1) * 128]
                nc.tensor.matmul(ps[:, 0:512], lhs, wv_sb[d][:, 0:512],
                                 start=(d == 0), stop=(d == DT - 1),
                                 skip_group_check=True)
                nc.tensor.matmul(ps[:, 512:768], lhs, wv_sb[d][:, 512:768],
                                 start=(d == 0), stop=(d == DT - 1),
                                 skip_group_check=True)
            vt = vsb_pool.tile([128, H * (HD + 1)], MMDT, tag=f"vsb{tt}")
            vt3 = vt[:].rearrange("p (h s) -> p h s", s=HD + 1)
            nc.vector.tensor_copy(
                vt3[:, :, 0:HD],
                ps[:, 0:D].rearrange("p (h s) -> p h s", s=HD))
            nc.vector.tensor_copy(
                vt3[:, :, HD:HD + 1],
                onesr[:].rearrange("p (h o) -> p h o", o=1))
            vsb[tt] = vt

        for emit in k_chunks(0):
            emit()

        for j in range(DT):
            ot = ot_pool.tile([128, QS], MMDT, tag=f"ot{j}")
            # K_{j+1} chunk emitters, spread across this pair's kt steps
            # (for pair 0 only into head 1's steps — head 0's steps carry V')
            pending_k = list(k_chunks(j + 1)) if j + 1 < DT else []
            k_at = {}
            if pending_k:
                lo = KTc if j == 0 else 0
                span = 2 * KTc - lo
                for i, em in enumerate(pending_k):
                    pos = lo + (i + 1) * span // (len(pending_k) + 1)
                    assert pos not in k_at
                    k_at[pos] = em
            step = 0
            for hi in range(2):
                h = 2 * j + hi
                poff = hi * 64
                ops = [op_pool.tile([65, 512], F32, tag=f"op{c}",
                                    name=f"op{h}_{c}") for c in range(QC)]
                for kt in range(KTc):
                    if h == 0:
                        emit_v(kt)
                    if step in k_at:
                        k_at.pop(step)()
                    step += 1
                    sp = sp_pool.tile([128, QS], F32, tag="sp")
                    for c in range(QC):
                        nc.tensor.matmul(
                            sp[:, c * 512:(c + 1) * 512],
                            ksb[j][poff:poff + 64, kt * 128:(kt + 1) * 128],
                            qts[j][poff:poff + 64, c * 512:(c + 1) * 512],
                            start=True, stop=True, skip_group_check=True)
                    p = p_pool.tile([128, QS], MMDT, tag="p")
                    nc.scalar.activation(p[:], sp[:], AF.Exp,
                                         bias=mb_sb[:, kt:kt + 1], scale=0.125)
                    for c in range(QC):
                        nc.tensor.matmul(
                            ops[c][:],
                            vsb[kt][:, h * (HD + 1):(h + 1) * (HD + 1)],
                            p[:, c * 512:(c + 1) * 512],
                            start=(kt == 0), stop=(kt == KTc - 1))
                for c in range(QC):
                    recip = nrm_pool.tile([1, 512], F32, tag="recip")
                    nc.vector.reciprocal(recip[:], ops[c][64:65, :])
                    bc_sb = nrm_pool.tile([64, 512], F32, tag="bc_sb")
                    nc.gpsimd.partition_broadcast(bc_sb[:], recip[:])
                    nc.vector.tensor_mul(
                        ot[poff:poff + 64, c * 512:(c + 1) * 512],
                        ops[c][0:64, :], bc_sb[:])
            for pos in sorted(k_at):       # any chunks not yet emitted
                k_at.pop(pos)()
            ots.append(ot)
    return ots


def _phase_c(nc, tc, ots, wp_sb, bp_sb, outT):
    from contextlib import ExitStack

    with ExitStack() as s:
        ost_pool = s.enter_context(tc.tile_pool(name="ost", bufs=3))
        pps_pool = s.enter_context(
            tc.tile_pool(name="pps", bufs=2, space="PSUM"))

        for m in range(DT):
            for c in range(QC):
                ps = pps_pool.tile([128, 512], F32, tag="pps")
                for j in range(DT):
                    nc.tensor.matmul(
                        ps[:], wp_sb[j][:, m * 128:(m + 1) * 128],
                        ots[j][:, c * 512:(c + 1) * 512],
                        start=(j == 0), stop=(j == DT - 1))
                ost = ost_pool.tile([128, 512], F32, tag="ost")
                nc.vector.tensor_scalar_add(ost[:], ps[:], bp_sb[:, m:m + 1])
                nc.sync.dma_start(
                    outT[m * 128:(m + 1) * 128, c * 512:(c + 1) * 512],
                    ost[:])


# ---------------------------------------------------------------- host side

@functools.lru_cache(maxsize=None)
def _get_runner(reps: int = 1, n_pad: int = N_PAD_DEFAULT):
    import jax
    from jax.sharding import Mesh, PartitionSpec
    from jax.experimental.shard_map import shard_map

    nc = build_program(reps, n_pad)
    install_neuronx_cc_hook()
    partition_name = (nc.partition_id_tensor.name
                      if nc.partition_id_tensor else None)
    in_names, out_names, out_avals, out_shapes = [], [], [], []
    for alloc in nc.m.functions[0].allocations:
        if not isinstance(alloc, mybir.MemoryLocationSet):
            continue
        name = alloc.memorylocations[0].name
        if alloc.kind == "ExternalInput":
            if name != partition_name:
                in_names.append(name)
        elif alloc.kind == "ExternalOutput":
            out_names.append(name)
            shape = tuple(alloc.tensor_shape)
            dtype = mybir.dt.np(alloc.dtype)
            out_avals.append(jax.core.ShapedArray(shape, dtype))
            out_shapes.append((shape, dtype))
    n_params = len(in_names)
    n_outs = len(out_avals)
    all_in_names = list(in_names) + list(out_names)
    if partition_name is not None:
        all_in_names.append(partition_name)
    donate = tuple(range(n_params, n_params + n_outs))

    def _bodyf(*args):
        operands = list(args)
        if partition_name is not None:
            operands.append(partition_id_tensor())
        outs = _bass_exec_p.bind(
            *operands,
            out_avals=tuple(out_avals),
            in_names=tuple(all_in_names),
            out_names=tuple(out_names),
            lowering_input_output_aliases=(),
            sim_require_finite=True,
            sim_require_nnan=True,
            nc=nc,
        )
        return tuple(outs)

    devices = jax.devices()[:N_CORES]
    mesh = Mesh(np.asarray(devices), ("core",))
    in_specs = (PartitionSpec("core"),) * (n_params + n_outs)
    out_specs = (PartitionSpec("core"),) * len(out_names)
    sharded = jax.jit(
        shard_map(_bodyf, mesh=mesh, in_specs=in_specs, out_specs=out_specs,
                  check_rep=False),
        donate_argnums=donate, keep_unused=True,
    )

    def run(in_maps):
        import jax as _jax
        per_core = [[np.asarray(m[n]) for n in in_names] for m in in_maps]
        concat_in = [np.concatenate([per_core[c][i] for c in range(N_CORES)],
                                    axis=0) for i in range(n_params)]
        concat_zeros = [np.zeros((N_CORES * s[0], *s[1:]), dt)
                        for (s, dt) in out_shapes]
        out_arrs = sharded(*concat_in, *concat_zeros)
        _jax.block_until_ready(out_arrs)
        return [
            {name: np.asarray(out_arrs[i]).reshape(
                N_CORES, *out_shapes[i][0])[c]
             for i, name in enumerate(out_names)}
            for c in range(N_CORES)
        ]

    return run


def make_in_maps(x, mask, w_qkv, w_proj, b_proj):
    import ml_dtypes
    mm_np = ml_dtypes.bfloat16 if USE_BF16 else np.float32
    x = np.asarray(x, np.float32)
    mask = np.asarray(mask, bool)
    w_qkv = np.asarray(w_qkv, np.float32)
    w_proj = np.asarray(w_proj, np.float32)
    b_proj = np.asarray(b_proj, np.float32)

    n_pad = _n_pad_from_mask(mask)
    KTc = n_pad // 128

    wqT = np.ascontiguousarray(w_qkv[0:D].T).astype(mm_np)
    wkT = np.ascontiguousarray(w_qkv[D:2 * D].T).astype(mm_np)
    wvT = np.ascontiguousarray(w_qkv[2 * D:3 * D].T).astype(mm_np)
    wpT = np.ascontiguousarray(w_proj.T).astype(mm_np)
    bp = np.ascontiguousarray(b_proj.reshape(DT, 128).T)
    onesc = np.ones((128, H), mm_np)

    xTs, xTcs, mbs = [], [], []
    for b in range(B):
        xT = np.ascontiguousarray(x[b].T).astype(mm_np)
        xTs.append(xT)
        idx = np.flatnonzero(~mask[b])
        nb = len(idx)
        xTc = np.zeros((D, n_pad), mm_np)
        xTc[:, :nb] = xT[:, idx]
        xTcs.append(xTc)
        mb = np.full(n_pad, -np.float32(NB), np.float32)
        mb[:nb] = 0.0
        mbs.append(np.ascontiguousarray(mb.reshape(KTc, 128).T))

    in_maps = []
    for c in range(N_CORES):
        b, qi = divmod(c, CORES_PER_B)
        q0 = qi * QS
        in_maps.append({
            "xTq": np.ascontiguousarray(xTs[b][:, q0:q0 + QS]),
            "xTc": xTcs[b],
            "wqT": wqT, "wkT": wkT, "wvT": wvT, "wpT": wpT,
            "bp": bp, "mbc": mbs[b], "onesc": onesc,
        })
    return in_maps


def assemble_output(results):
    out = np.empty((B, T, D), np.float32)
    for c in range(N_CORES):
        b, qi = divmod(c, CORES_PER_B)
        q0 = qi * QS
        out[b, q0:q0 + QS, :] = results[c]["outT"].T
    return out


def kernel(x, mask, w_qkv, w_proj, b_proj):
    n_pad = _n_pad_from_mask(mask)
    run = _get_runner(1, n_pad)
    in_maps = make_in_maps(x, mask, w_qkv, w_proj, b_proj)
    results = run(in_maps)
    return assemble_output(results)
